# revision 6
# baseline (speedup 1.0000x reference)
"""GCN (GCNConv + 3-layer MLP + log_softmax) on 8 Trainium2 NeuronCores.

Strategy (pull-mode message passing):
  - Nodes are sharded 8 ways by destination; each core owns 12500 dst nodes
    (padded to 12544 = 98 tiles of 128).
  - Every core computes the full transformed feature table h = x @ W_gcn
    ([100352, 64] fp32, rows padded) into its own DRAM — replicating this
    small matmul is cheaper than an AllGather of h.
  - Edges (incl. self-loops) are partitioned by dst shard on the host,
    sorted by (dst tile, src group, src), padded to 128-edge chunks.
  - Per 128-edge chunk the core gathers h[src] rows with dma_gather
    (256 B/row), builds a scaled one-hot matrix S[e, j] = norm[e] *
    (dst_local[e] == j) with one fused tensor_scalar op, and accumulates
    aggT[64, 128] += msgs.T @ S on the tensor engine (PSUM).
  - The MLP runs in feature-major (transposed) layout so all biases are
    per-partition activation biases; the last matmul flips back to
    node-major and log_softmax finishes on [128, 4] tiles.
"""

import os
import sys

import numpy as np

sys.path.insert(0, "/opt/trn_rl_repo")

N = 100000
F = 256
H = 64
NCLS = 4
NCORES = 8
SHARD = 12500
SPAD = 12544          # 98 * 128
NT = SPAD // 128      # 98 dst tiles per core
NPAD = SPAD * NCORES  # 100352
NG = 4
GSZ = NPAD // NG      # 25088 rows per src group (< 2**15 for int16 idx)
TSB1 = 16             # phase-1 tiles per superblock (784 = 49*16 tiles)
TSB3 = 6              # phase-3 dst tiles per superblock


def _host_prep(edge_index):
    """Partition/sort/pad edges; returns per-core device arrays + meta."""
    src = np.asarray(edge_index[0]).astype(np.int64)
    dst = np.asarray(edge_index[1]).astype(np.int64)
    deg = np.bincount(dst, minlength=N).astype(np.float64) + 1.0
    dinv = 1.0 / np.sqrt(deg)

    loop = np.arange(N, dtype=np.int64)
    srcA = np.concatenate([src, loop])
    dstA = np.concatenate([dst, loop])
    norm = (dinv[srcA] * dinv[dstA]).astype(np.float32)

    core = dstA // SHARD
    dl = dstA - core * SHARD
    tl = dl >> 7
    dloc = (dl & 127).astype(np.float32)
    srcp = (srcA // SHARD) * SPAD + (srcA % SHARD)   # padded global src id
    # h_all rows are stored partition-major per phase-1 superblock (so the
    # h write DMA is contiguous): node srcp lives at h_all row perm(srcp).
    blk = TSB1 * 128
    b = srcp // blk
    r = srcp - b * blk
    srcp = b * blk + (r % 128) * TSB1 + r // 128
    grp = srcp // GSZ
    idx16 = (srcp - grp * GSZ).astype(np.int16)

    key = ((core * NT + tl) * NG + grp)
    order = np.argsort(key * np.int64(NPAD) + srcp, kind="stable")
    key_s = key[order]
    idx_s = idx16[order]
    dloc_s = dloc[order]
    norm_s = norm[order]

    cnt = np.bincount(key, minlength=NCORES * NT * NG).reshape(NCORES, NT, NG)
    C = ((cnt.max(axis=0) + 127) // 128).astype(np.int64)      # [NT, NG] chunks
    starts = np.zeros(NCORES * NT * NG + 1, dtype=np.int64)
    np.cumsum(cnt.reshape(-1), out=starts[1:])

    # superblock partition of the 98 tiles
    sbs = [list(range(s, min(s + TSB3, NT))) for s in range(0, NT, TSB3)]
    # stream layout: for sb: for g: for t in sb: C[t,g] chunks of 128 edges
    col_of = np.zeros((NT, NG), dtype=np.int64)   # chunk column of (t, g)
    sb_meta = []
    col = 0
    for tiles in sbs:
        colbase = col
        Ls = []
        goffs = []
        for g in range(NG):
            goffs.append(col - colbase)
            for t in tiles:
                col_of[t, g] = col
                col += C[t, g]
            Ls.append(int(128 * sum(C[t, g] for t in tiles)))
        sb_meta.append(dict(tiles=tiles, colbase=int(colbase),
                            totc=int(col - colbase), L=Ls, goff=goffs))
    TOTC = int(col)
    TOT = TOTC * 128

    idx_streams, dloc_streams, norm_streams = [], [], []
    for c in range(NCORES):
        si = np.zeros(TOT, dtype=np.int16)
        sd = np.full(TOT, -1.0, dtype=np.float32)
        sn = np.zeros(TOT, dtype=np.float32)
        for t in range(NT):
            for g in range(NG):
                k = (c * NT + t) * NG + g
                n = cnt[c, t, g]
                if n == 0:
                    continue
                a = starts[k]
                o = col_of[t, g] * 128
                si[o:o + n] = idx_s[a:a + n]
                sd[o:o + n] = dloc_s[a:a + n]
                sn[o:o + n] = norm_s[a:a + n]
        idx_streams.append(np.tile(si.reshape(-1, 16).T, (8, 1)))      # [128, TOT/16]
        dloc_streams.append(np.ascontiguousarray(sd.reshape(-1, 128).T))  # [128, TOTC]
        norm_streams.append(np.ascontiguousarray(sn.reshape(-1, 128).T))
    meta = dict(C=C, sb_meta=sb_meta, TOTC=TOTC, TOT=TOT)
    return idx_streams, dloc_streams, norm_streams, meta


def _build_nc(meta):
    import concourse.bacc as bacc
    import concourse.mybir as mybir
    import concourse.tile as tile
    from concourse import library_config

    f32 = mybir.dt.float32
    i16 = mybir.dt.int16
    AF = mybir.ActivationFunctionType
    ALU = mybir.AluOpType
    TOTC, TOT = meta["TOTC"], meta["TOT"]
    C, sb_meta = meta["C"], meta["sb_meta"]

    nc = bacc.Bacc("TRN2")
    xT = nc.dram_tensor("xT", [F, NPAD], f32, kind="ExternalInput")
    wg = nc.dram_tensor("wg", [F, H], f32, kind="ExternalInput")
    w1 = nc.dram_tensor("w1", [64, 32], f32, kind="ExternalInput")
    w2 = nc.dram_tensor("w2", [32, 16], f32, kind="ExternalInput")
    w3 = nc.dram_tensor("w3", [16, 4], f32, kind="ExternalInput")
    bg = nc.dram_tensor("bg", [64, 1], f32, kind="ExternalInput")
    b1 = nc.dram_tensor("b1", [32, 1], f32, kind="ExternalInput")
    b2 = nc.dram_tensor("b2", [16, 1], f32, kind="ExternalInput")
    b3r = nc.dram_tensor("b3r", [1, 4], f32, kind="ExternalInput")
    iotam = nc.dram_tensor("iotam", [128, 128], f32, kind="ExternalInput")
    onesr = nc.dram_tensor("onesr", [1, 128], f32, kind="ExternalInput")
    idxT = nc.dram_tensor("idx", [128, TOT // 16], i16, kind="ExternalInput")
    dlocT = nc.dram_tensor("dloc", [128, TOTC], f32, kind="ExternalInput")
    nrmT = nc.dram_tensor("nrm", [128, TOTC], f32, kind="ExternalInput")
    outT = nc.dram_tensor("out", [SPAD, NCLS], f32, kind="ExternalOutput")

    NT1 = NPAD // 128  # 784 phase-1 tiles
    sb1 = [list(range(s, min(s + TSB1, NT1))) for s in range(0, NT1, TSB1)]
    maxc = max(m["totc"] for m in sb_meta)

    with tile.TileContext(nc) as tc:
        with tc.tile_pool(name="const", bufs=1) as cp, \
             tc.tile_pool(name="dram", bufs=1, space="DRAM") as dram:
            h_all = dram.tile([NPAD, H], f32)
            nc.gpsimd.load_library(library_config.mlp)

            wg0 = cp.tile([128, H], f32, tag="wg0")
            wg1 = cp.tile([128, H], f32, tag="wg1")
            nc.sync.dma_start(wg0[:], wg[0:128, :])
            nc.sync.dma_start(wg1[:], wg[128:256, :])
            w1s = cp.tile([64, 32], f32, tag="w1s")
            w2s = cp.tile([32, 16], f32, tag="w2s")
            w3s = cp.tile([16, 4], f32, tag="w3s")
            bgs = cp.tile([64, 1], f32, tag="bgs")
            b1s = cp.tile([32, 1], f32, tag="b1s")
            b2s = cp.tile([16, 1], f32, tag="b2s")
            b3s = cp.tile([1, 4], f32, tag="b3s")
            iots = cp.tile([128, 128], f32, tag="iots")
            ones = cp.tile([1, 128], f32, tag="ones")
            for t_, d_ in ((w1s, w1), (w2s, w2), (w3s, w3), (bgs, bg),
                           (b1s, b1), (b2s, b2), (b3s, b3r), (iots, iotam),
                           (ones, onesr)):
                nc.sync.dma_start(t_[:], d_[:, :])

            # ---------------- phase 1: h = x @ W_gcn (full, replicated) ----
            with tc.tile_pool(name="p1", bufs=2) as p1p, \
                 tc.tile_pool(name="ps1", bufs=4, space="PSUM") as ps1:
                for tiles in sb1:
                    T = len(tiles)
                    t0 = tiles[0]
                    xt0 = p1p.tile([128, TSB1 * 128], f32, tag="xt0")
                    xt1 = p1p.tile([128, TSB1 * 128], f32, tag="xt1")
                    nc.sync.dma_start(
                        xt0[:, :T * 128], xT[0:128, t0 * 128:(t0 + T) * 128])
                    nc.sync.dma_start(
                        xt1[:, :T * 128], xT[128:256, t0 * 128:(t0 + T) * 128])
                    hsb = p1p.tile([128, TSB1, H], f32, tag="hsb")
                    for i in range(T):
                        ps = ps1.tile([128, H], f32, tag="hps")
                        nc.tensor.matmul(ps[:], xt0[:, i * 128:(i + 1) * 128],
                                         wg0[:], start=True, stop=False)
                        nc.tensor.matmul(ps[:], xt1[:, i * 128:(i + 1) * 128],
                                         wg1[:], start=False, stop=True)
                        nc.vector.tensor_copy(hsb[:, i, :], ps[:])
                    # partition-major row order -> contiguous 4 KB runs
                    nc.sync.dma_start(
                        h_all[t0 * 128:(t0 + T) * 128, :]
                        .rearrange("(p t) f -> p t f", p=128),
                        hsb[:, :T, :])

            # ---------------- phase 3: gather + aggregate + MLP ------------
            with tc.tile_pool(name="p3", bufs=2) as p3p, \
                 tc.tile_pool(name="gb", bufs=2) as gbp, \
                 tc.tile_pool(name="sp", bufs=6) as sp, \
                 tc.tile_pool(name="ep", bufs=3) as ep, \
                 tc.tile_pool(name="oa", bufs=1) as oap, \
                 tc.tile_pool(name="agg", bufs=3, space="PSUM") as aggp, \
                 tc.tile_pool(name="mlp", bufs=3, space="PSUM") as mlpp:
                outacc = oap.tile([128, NT, NCLS], f32, tag="outacc")
                for m in sb_meta:
                    tiles, colbase, totc = m["tiles"], m["colbase"], m["totc"]
                    idxsb = p3p.tile([128, maxc * 8], i16, tag="idx")
                    nc.sync.dma_start(idxsb[:, :totc * 8],
                                      idxT[:, colbase * 8:(colbase + totc) * 8])
                    dlsb = p3p.tile([128, maxc], f32, tag="dl")
                    nrsb = p3p.tile([128, maxc], f32, tag="nr")
                    nc.sync.dma_start(dlsb[:, :totc],
                                      dlocT[:, colbase:colbase + totc])
                    nc.sync.dma_start(nrsb[:, :totc],
                                      nrmT[:, colbase:colbase + totc])
                    gbuf = gbp.tile([128, maxc, H], f32, tag="gbuf")
                    for g in range(NG):
                        L = m["L"][g]
                        go = m["goff"][g]
                        # SWDGE descriptor ring caps one gather at ~1024 idxs
                        for k in range(0, L, 1024):
                            ni = min(1024, L - k)
                            c0 = go + k // 128
                            nc.gpsimd.dma_gather(
                                gbuf[:, c0:c0 + ni // 128, :],
                                h_all[g * GSZ:(g + 1) * GSZ, :],
                                idxsb[:, c0 * 8:(c0 + ni // 128) * 8],
                                ni, ni, H)
                    for ti, t in enumerate(tiles):
                        agg = aggp.tile([64, 128], f32, tag="agg")
                        nch = int(C[t].sum())
                        done = 0
                        for g in range(NG):
                            base = m["goff"][g] + int(
                                sum(C[tt, g] for tt in tiles[:ti]))
                            for j in range(int(C[t, g])):
                                pos = base + j
                                S = sp.tile([128, 128], f32, tag="S")
                                nc.vector.tensor_scalar(
                                    S[:], iots[:], dlsb[:, pos:pos + 1],
                                    nrsb[:, pos:pos + 1],
                                    op0=ALU.is_equal, op1=ALU.mult)
                                nc.tensor.matmul(
                                    agg[:], gbuf[:, pos, :], S[:],
                                    start=(done == 0), stop=(done == nch - 1))
                                done += 1
                        t0s = ep.tile([64, 128], f32, tag="t0")
                        nc.scalar.activation(t0s[:], agg[:], AF.Relu,
                                             bias=bgs[:])
                        pm1 = mlpp.tile([32, 128], f32, tag="pm")
                        nc.tensor.matmul(pm1[:], w1s[:], t0s[:],
                                         start=True, stop=True)
                        t1s = ep.tile([32, 128], f32, tag="t1")
                        nc.scalar.activation(t1s[:], pm1[:], AF.Relu,
                                             bias=b1s[:])
                        pm2 = mlpp.tile([16, 128], f32, tag="pm")
                        nc.tensor.matmul(pm2[:], w2s[:], t1s[:],
                                         start=True, stop=True)
                        t2s = ep.tile([16, 128], f32, tag="t2")
                        nc.scalar.activation(t2s[:], pm2[:], AF.Relu,
                                             bias=b2s[:])
                        pm3 = mlpp.tile([128, NCLS], f32, tag="pm")
                        nc.tensor.matmul(pm3[:], t2s[:], w3s[:],
                                         start=True, stop=False)
                        nc.tensor.matmul(pm3[:], ones[:], b3s[:],
                                         start=False, stop=True)
                        nmax = ep.tile([128, 1], f32, tag="nmax")
                        nc.vector.tensor_reduce(nmax[:], pm3[:],
                                                axis=mybir.AxisListType.X,
                                                op=ALU.max, negate=True)
                        esb = ep.tile([128, NCLS], f32, tag="esb")
                        ssum = ep.tile([128, 1], f32, tag="ssum")
                        nc.scalar.activation(esb[:], pm3[:], AF.Exp,
                                             bias=nmax[:], accum_out=ssum[:])
                        lsb = ep.tile([128, 1], f32, tag="lsb")
                        nc.scalar.activation(lsb[:], ssum[:], AF.Ln)
                        nc.vector.tensor_scalar(
                            outacc[:, t, :], pm3[:], nmax[:], lsb[:],
                            op0=ALU.add, op1=ALU.subtract)
                nc.sync.dma_start(
                    outT[:, :].rearrange("(t p) c -> p t c", p=128),
                    outacc[:])
    nc.compile()
    return nc


def kernel(x, edge_index, W_gcn, b_gcn, W1, b1, W2, b2, W3, b3,
           _trace=False):
    from concourse.bass_utils import run_bass_kernel_spmd

    x = np.asarray(x, dtype=np.float32)
    idx_streams, dloc_streams, norm_streams, meta = _host_prep(edge_index)
    nc = _build_nc(meta)

    xTp = np.zeros((F, NPAD), dtype=np.float32)
    xt = np.ascontiguousarray(x.T)
    for c in range(NCORES):
        xTp[:, c * SPAD:c * SPAD + SHARD] = xt[:, c * SHARD:(c + 1) * SHARD]
    common = {
        "xT": xTp,
        "wg": np.asarray(W_gcn, np.float32),
        "w1": np.asarray(W1, np.float32),
        "w2": np.asarray(W2, np.float32),
        "w3": np.asarray(W3, np.float32),
        "bg": np.asarray(b_gcn, np.float32).reshape(64, 1),
        "b1": np.asarray(b1, np.float32).reshape(32, 1),
        "b2": np.asarray(b2, np.float32).reshape(16, 1),
        "b3r": np.asarray(b3, np.float32).reshape(1, 4),
        "iotam": np.tile(np.arange(128, dtype=np.float32), (128, 1)),
        "onesr": np.ones((1, 128), dtype=np.float32),
    }
    in_maps = []
    for c in range(NCORES):
        m = dict(common)
        m["idx"] = idx_streams[c]
        m["dloc"] = dloc_streams[c]
        m["nrm"] = norm_streams[c]
        in_maps.append(m)

    res = run_bass_kernel_spmd(nc, in_maps, core_ids=list(range(NCORES)),
                               trace=_trace)
    out = np.concatenate(
        [res.results[c]["out"][:SHARD] for c in range(NCORES)], axis=0)
    if _trace:
        kernel.last_exec_time_ns = res.exec_time_ns
    return out


kernel.last_exec_time_ns = None
